# revision 46
# baseline (speedup 1.0000x reference)
"""Distributed Trainium2 Bass kernel for multi-head attention w/ RoPE.

Reference op (B=4, S=2048, D=1024, H=16, HD=64, fp32):
    q/k/v = hidden @ W{q,k,v}.T + b   (per-head reshape)
    q, k  = rope(q), rope(k)
    out   = softmax(q k^T / sqrt(HD)) v  @ Wo.T

Sharding: 8 cores = 4 batches x 2 query-halves. Each core projects Q
for its own 1024 queries and computes V (and pi0/pi1's K) for BOTH
pair halves locally from xaT/xbT (the pair's x in global slot order --
same data on both pair cores, keeping the SPMD program uniform); K for
pi>=2 is projected for the own half and pairwise-AllGathered. Nothing
before pi0's attention depends on a collective (the CC engine takes
~25us to boot and its early ops run 10-50us). Host unshard is a concat.

Schedule (PE-issue-order is emission order; every stall class found in
the traces has a structural fix here):
  * Unified 3-slot PSUM ring of [128,1024] f32 tiles (6 banks) carries
    every projection / score tile; attn@V accumulators take the last 2
    banks. The ring slack decouples score production from softmax-exp
    (ACT), the steady-state pacer.
  * ACT runs *only* the exp stream (1 elem/cycle/lane is its hard
    floor, ~255us total). RoPE band swaps ride the gpsimd DMA queue;
    all PSUM evictions are DVE; output is stored bf16.
  * Even/odd-head score matmuls issue adjacently on disjoint 64-row PE
    groups and disjoint PSUM banks -> the pair runs CONCURRENTLY in
    the array (measured dt~4ns), halving score time.
  * Projection work is sliced into filler thunks (one 512-token chunk
    + its PSUM-freeing cos-mul) popped between kcp groups INSIDE the
    attention stream, and the remaining V-proj blocks fill later
    bodies, so ACT's exp pipeline never drains at phase boundaries.
  * The last attn@V burst of each query-half (which waits on the final
    exps) is always deferred into the next block's kcp0.
  * RoPE sin term is recovered as (q*cos)*tan -- the tan table is
    sign-folded sin/cos -- so the PSUM tile has a single DVE reader
    and the fp32 read happens once.
All matmuls bf16 with fp32 accumulation; exp folds the 1/sqrt(HD)
scale; the softmax denominator rides an appended ones-column through
attn@V and normalization runs 4-stage (hop/recip/bcast/mul) off the
critical path.
"""

import sys

import numpy as np

try:  # concourse ships in the container; fall back to the staged repo
    import concourse.bass  # noqa: F401
except Exception:  # pragma: no cover
    sys.path.insert(0, "/opt/trn_rl_repo")

import ml_dtypes

B, S, D, H = 4, 2048, 1024, 16
HD = D // H                      # 64
P = 128
NCORES = 8
SQ = S // 2                      # 1024 queries per core
SK = S                           # 2048 keys per core
ND = D // P                      # 8 feature chunks
NT = SK // P                     # 16 key/token chunks
QF = 512                         # matmul moving width
NQF = SQ // QF                   # 2
ROPE_BASE = 10000.0
BF16 = ml_dtypes.bfloat16

TRACE = False                    # test harness flips this
TRACE_KW = {}
LAST = {}                        # exec_time_ns / trace path for test harness

_cache = {}


def _build_nc(with_bias):
    import concourse.bass as bass
    import concourse.mybir as mybir
    import concourse.tile as tile
    from concourse import bacc
    from contextlib import ExitStack

    f32 = mybir.dt.float32
    bf16 = mybir.dt.bfloat16
    AF = mybir.ActivationFunctionType
    PSUM = bass.MemorySpace.PSUM

    nc = bacc.Bacc(None)
    # xT: own query half; xaT/xbT: the PAIR's two halves in global slot
    # order (identical data on both pair cores -- keeps the SPMD program
    # uniform while local K/V match the gathered-K slot order)
    xT = nc.declare_dram_parameter("xT", [D + 1, SQ], bf16, False)
    xaT = nc.declare_dram_parameter("xaT", [D + 1, SQ], bf16, False)
    xbT = nc.declare_dram_parameter("xbT", [D + 1, SQ], bf16, False)
    wqT = nc.declare_dram_parameter("wqT", [D + 1, D], bf16, False)
    wkT = nc.declare_dram_parameter("wkT", [D + 1, D], bf16, False)
    wvT = nc.declare_dram_parameter("wvT", [D + 1, D], bf16, False)
    woT = nc.declare_dram_parameter("woT", [D, D], bf16, False)
    cosk = nc.declare_dram_parameter("cosk", [P, SQ], bf16, False)
    sink = nc.declare_dram_parameter("sink", [P, SQ], bf16, False)
    cosa = nc.declare_dram_parameter("cosa", [P, SQ], bf16, False)
    sina = nc.declare_dram_parameter("sina", [P, SQ], bf16, False)
    cosb = nc.declare_dram_parameter("cosb", [P, SQ], bf16, False)
    sinb = nc.declare_dram_parameter("sinb", [P, SQ], bf16, False)
    out = nc.declare_dram_parameter("out", [SQ, D], bf16, True)
    RG = [[0, 1], [2, 3], [4, 5], [6, 7]]
    BYP = mybir.AluOpType.bypass
    # HBM staging for the pair-wise K AllGathers (V is computed locally
    # from the partner's x half -- no V collectives at all)
    kstg = [nc.dram_tensor(f"kstg{i}", [P, SQ], bf16) for i in range(ND)]
    kgth = [nc.dram_tensor(f"kgth{i}", [2, P, SQ], bf16) for i in range(ND)]
    ccw_in = nc.dram_tensor("ccw_in", [1, 64], bf16)
    ccw_out = nc.dram_tensor("ccw_out", [2, 1, 64], bf16)

    with tile.TileContext(nc) as tc, ExitStack() as st:
        sb = st.enter_context(tc.tile_pool(name="sb", bufs=1))
        qk = st.enter_context(tc.tile_pool(name="qk", bufs=3))
        wp = st.enter_context(tc.tile_pool(name="wp", bufs=2))
        tp = st.enter_context(tc.tile_pool(name="tp", bufs=2))
        etp = st.enter_context(tc.tile_pool(name="et", bufs=5))
        npool = st.enter_context(tc.tile_pool(name="nrm", bufs=3))
        outp = st.enter_context(tc.tile_pool(name="ou", bufs=1))
        psu = st.enter_context(tc.tile_pool(name="psu", bufs=3, space=PSUM))
        pso = st.enter_context(tc.tile_pool(name="pso", bufs=2, space=PSUM))

        def u_slot(nm):
            # one ring slot: [128, 1024] f32 = 2 PSUM banks, 3-deep ring
            return psu.tile([P, SQ], f32, tag="u", name=nm)

        at = [sb.tile([P, SQ], bf16, tag=f"at{i}", name=f"at{i}")
              for i in range(ND)]

        # ---- PE warm-up: dummy matmuls during the initial DMA wait -----
        wu = wp.tile([P, QF], bf16, tag="wu", name="wu", bufs=1)
        nc.vector.memset(wu[:], 0.0)
        psw = u_slot("psw")
        for i in range(14):
            nc.tensor.matmul(psw[:, 0:QF], wu[:, 0:P], wu[:],
                             start=(i == 0), stop=(i == 13))

        # ---- CC warm-up: absorbs the ~30us collective spin-up ----------
        ccwt = wp.tile([1, 64], bf16, tag="ccw", name="ccw", bufs=1)
        nc.vector.memset(ccwt[:], 0.0)
        nc.sync.dma_start(out=ccw_in[:, :], in_=ccwt[:])
        nc.gpsimd.collective_compute(
            "AllGather", BYP, replica_groups=RG,
            ins=[ccw_in[:, :]], outs=[ccw_out[:, :, :]])

        # ---- loads (first K/Q weight slices first so K proj can start
        # streaming behind the x chunks as they land) -------------------
        def load_wslice(wdram, wtag):
            ws = wp.tile([P, ND, P], bf16, tag=wtag, name=wtag)
            nc.sync.dma_start(
                out=ws[:],
                in_=wdram[0:D, :].rearrange("(n p) o -> p n o", p=P))
            wb = None
            if with_bias:
                wb = wp.tile([1, P], bf16, tag=wtag + "b", name=wtag + "b")
                nc.sync.dma_start(out=wb[:], in_=wdram[D:D + 1, :])
            return ws, wb

        # V proj (first in PE order) contracts wv x (xa,xb): interleave
        # those DMAs so its d-chunk matmuls stream behind the arrivals;
        # xs (only needed by Q proj, later) loads after.
        wv = wp.tile([P, ND, D], bf16, tag="wbig", name="wv", bufs=1)
        xs = [sb.tile([P, SQ], bf16, tag=f"x{d}", name=f"x{d}")
              for d in range(ND)]
        xa = [sb.tile([P, SQ], bf16, tag=f"xa{d}", name=f"xa{d}")
              for d in range(ND)]
        xb = [sb.tile([P, SQ], bf16, tag=f"xb{d}", name=f"xb{d}")
              for d in range(ND)]
        for d_ in range(ND):
            nc.sync.dma_start(out=wv[:, d_, 0:4 * HD],
                              in_=wvT[d_ * P:(d_ + 1) * P, 0:4 * HD])
            nc.sync.dma_start(out=xa[d_][:], in_=xaT[d_ * P:(d_ + 1) * P, :])
            nc.sync.dma_start(out=xb[d_][:], in_=xbT[d_ * P:(d_ + 1) * P, :])
        cks = {}
        for nm, src in (("ck", cosk), ("sk", sink), ("cka", cosa),
                        ("ska", sina), ("ckb", cosb), ("skb", sinb)):
            cks[nm] = sb.tile([P, SQ], bf16, tag=nm, name=nm)
            nc.sync.dma_start(out=cks[nm][:], in_=src[:, :])
        ck, sk_ = cks["ck"], cks["sk"]
        cka, ska, ckb, skb = cks["cka"], cks["ska"], cks["ckb"], cks["skb"]
        wsl = {0: (load_wslice(wqT[:, 0:P], "wq"), load_wslice(wkT[:, 0:P], "wk")),
               1: (load_wslice(wqT[:, P:2 * P], "wq"),
                   load_wslice(wkT[:, P:2 * P], "wk"))}
        for d_ in range(ND):
            nc.sync.dma_start(out=xs[d_][:], in_=xT[d_ * P:(d_ + 1) * P, :])
        for d_ in range(ND):
            nc.sync.dma_start(out=wv[:, d_, 4 * HD:D],
                              in_=wvT[d_ * P:(d_ + 1) * P, 4 * HD:D])
        xone = xaone = xbone = None
        if with_bias:
            xone = sb.tile([1, SQ], bf16, tag="xone", name="xone")
            nc.sync.dma_start(out=xone[:], in_=xT[D:D + 1, :])
            xaone = sb.tile([1, SQ], bf16, tag="xaone", name="xaone")
            nc.sync.dma_start(out=xaone[:], in_=xaT[D:D + 1, :])
            xbone = sb.tile([1, SQ], bf16, tag="xbone", name="xbone")
            nc.sync.dma_start(out=xbone[:], in_=xbT[D:D + 1, :])
            wvb = wp.tile([1, D], bf16, tag="wvb", name="wvb", bufs=1)
            nc.sync.dma_start(out=wvb[:], in_=wvT[D:D + 1, :])

        def qk_proj(wsb, dst, xv, ctbl, stbl, xo):
            """dst[o128, 0:SQ] = rope(W[pi-slice] @ xv^T + b). The cos mul
            (the only PSUM reader) frees the u-slot after ONE DVE op; the
            sin term is recovered as dst * tan (stbl holds the
            sign-folded sin/cos table), an all-SBUF bf16 mul at the DVE
            fast rate. The 2-pi pipeline hides all the swap latency."""
            ws, wb = wsb
            ups = u_slot("ups")
            for c in range(NQF):
                cs = slice(c * QF, (c + 1) * QF)
                for d_ in range(ND):
                    nc.tensor.matmul(
                        ups[:, cs], ws[:, d_, :], xv[d_][:, cs],
                        start=(d_ == 0), stop=(not with_bias and d_ == ND - 1))
                if with_bias:
                    nc.tensor.matmul(
                        ups[:, cs], wb[:], xo[:, cs],
                        start=False, stop=True)
            t2 = tp.tile([P, SQ], bf16, tag="t2", name="t2")
            t2s = tp.tile([P, SQ], bf16, tag="t2s", name="t2s")
            nc.vector.tensor_mul(dst, ups[:], ctbl[:])
            nc.vector.tensor_mul(t2[:], dst, stbl[:])
            for b0 in (0, 64):
                nc.gpsimd.dma_start(out=t2s[b0:b0 + 32, :],
                                    in_=t2[b0 + 32:b0 + 64, :])
                nc.gpsimd.dma_start(out=t2s[b0 + 32:b0 + 64, :],
                                    in_=t2[b0:b0 + 32, :])
            nc.vector.tensor_add(dst, dst, t2s[:])

        kts, qts, vps = {}, {}, {}

        def qk_chunks(wsb, dtile, dbase, xv, ctbl, stbl, xo, post=None):
            """Split projection: two filler thunks, one per 512-token
            chunk. Each allocates its own (half-used) u-slot, freed by
            its cos-mul; the second finishes rope (tan mul + band swap +
            add) and runs `post` (K staging). Emitted INSIDE the
            attention stream so ACT's exp pipeline never drains."""
            ws, wb = wsb

            def chunk(c, fin):
                def run():
                    ups = u_slot("ups")
                    cs = slice(c * QF, (c + 1) * QF)
                    for d_ in range(ND):
                        nc.tensor.matmul(
                            ups[:, 0:QF], ws[:, d_, :], xv[d_][:, cs],
                            start=(d_ == 0),
                            stop=(not with_bias and d_ == ND - 1))
                    if with_bias:
                        nc.tensor.matmul(
                            ups[:, 0:QF], wb[:], xo[:, cs],
                            start=False, stop=True)
                    dsl = dtile[:, dbase + c * QF:dbase + (c + 1) * QF]
                    nc.vector.tensor_mul(dsl, ups[:, 0:QF], ctbl[:, cs])
                    if fin:
                        dst = dtile[:, dbase:dbase + SQ]
                        t2 = tp.tile([P, SQ], bf16, tag="t2", name="t2")
                        t2s = tp.tile([P, SQ], bf16, tag="t2s", name="t2s")
                        nc.vector.tensor_mul(t2[:], dst, stbl[:])
                        for b0 in (0, 64):
                            nc.gpsimd.dma_start(out=t2s[b0:b0 + 32, :],
                                                in_=t2[b0 + 32:b0 + 64, :])
                            nc.gpsimd.dma_start(out=t2s[b0 + 32:b0 + 64, :],
                                                in_=t2[b0:b0 + 32, :])
                        nc.vector.tensor_add(dst, dst, t2s[:])
                        if post is not None:
                            post()
                return run
            return [chunk(0, False), chunk(1, True)]

        def k_fill(pi_):
            kt_ = qk.tile([P, SK], bf16, tag="kt", name="kt", bufs=3)
            kts[pi_] = kt_

            def post():
                nc.sync.dma_start(out=kstg[pi_][:, :], in_=kt_[:, 0:SQ])
                nc.gpsimd.collective_compute(
                    "AllGather", BYP, replica_groups=RG,
                    ins=[kstg[pi_][:, :]], outs=[kgth[pi_][:, :, :]])
            return qk_chunks(wsl[pi_][1], kt_, 0, xs, ck, sk_, xone, post)

        def q_fill(pi_):
            qt_ = qk.tile([P, SQ], bf16, tag="qt", name="qt", bufs=3)
            qts[pi_] = qt_
            return qk_chunks(wsl[pi_][0], qt_, 0, xs, ck, sk_, xone)

        def k_local(pi_):
            # pi0/pi1: project BOTH pair-halves locally in slot order --
            # no collective gates the startup
            kt_ = qk.tile([P, SK], bf16, tag="kt", name="kt", bufs=3)
            qk_proj(wsl[pi_][1], kt_[:, 0:SQ], xa, cka, ska, xaone)
            qk_proj(wsl[pi_][1], kt_[:, SQ:SK], xb, ckb, skb, xbone)
            kts[pi_] = kt_

        def q_stage(pi_):
            qt_ = qk.tile([P, SQ], bf16, tag="qt", name="qt", bufs=3)
            qk_proj(wsl[pi_][0], qt_[:], xs, ck, sk_, xone)
            qts[pi_] = qt_

        def load_ktile(pi_):
            for s_ in range(2):
                nc.sync.dma_start(out=kts[pi_][:, s_ * SQ:(s_ + 1) * SQ],
                                  in_=kgth[pi_][s_, :, :])

        # V for ALL heads and BOTH pair-halves lives in one big SBUF
        # tile, computed locally from xa/xb in global slot order (so it
        # agrees with both the local pi0/pi1 K tiles and the gathered-K
        # slot order). No V collectives exist at all.
        vbig = sb.tile([P, 2, NT // 2, H, HD + 1], bf16, tag="vbig",
                       name="vbig")
        nc.vector.memset(vbig[:, :, :, :, HD:HD + 1], 1.0)

        def v_block(s_, tp_, h0, nh):
            """V proj for heads [h0, h0+nh) x token chunks (2tp_, 2tp_+1)
            of pair-half s_: one u-slot, two nh*64-wide accum groups."""
            xv, xo = (xa, xaone) if s_ == 0 else (xb, xbone)
            hw = nh * HD
            ups = u_slot("vps")
            for ti in range(2):
                t_ = 2 * tp_ + ti
                tqs = slice(ti * hw, (ti + 1) * hw)
                ohs = slice(h0 * HD, h0 * HD + hw)
                for d_ in range(ND):
                    nc.tensor.matmul(
                        ups[:, tqs], xv[d_][:, t_ * P:(t_ + 1) * P],
                        wv[:, d_, ohs],
                        start=(d_ == 0),
                        stop=(not with_bias and d_ == ND - 1))
                if with_bias:
                    nc.tensor.matmul(
                        ups[:, tqs], xo[:, t_ * P:(t_ + 1) * P],
                        wvb[:, ohs], start=False, stop=True)
            nc.vector.tensor_copy(
                vbig[:, s_, 2 * tp_:2 * tp_ + 2, h0:h0 + nh, 0:HD],
                ups[:, 0:2 * hw].rearrange("p (t h d) -> p t h d",
                                           h=nh, d=HD))

        # ---- normalization stage machinery (off the critical path) ----
        pending = []

        def norm_hops(batch):
            for ent in batch:
                den = npool.tile([1, QF], f32, tag="den", name="den", bufs=4)
                nc.gpsimd.dma_start(out=den[:], in_=ent[3][HD:HD + 1, :])
                ent.append(den)

        def norm_recips(batch):
            for ent in batch:
                rc = npool.tile([1, QF], f32, tag="rc", name="rc", bufs=4)
                nc.vector.reciprocal_approx_fast(rc[:], ent[4][:])
                ent.append(rc)

        def norm_bcasts(batch):
            for ent in batch:
                bc = npool.tile([HD, QF], f32, tag="bc", name="bc", bufs=3)
                nc.gpsimd.partition_broadcast(bc[:], ent[5][:])
                ent.append(bc)

        def norm_muls(batch):
            for h, ppi, qqs, osb, den, rc, bc in batch:
                if h % 2 == 0:
                    nc.vector.tensor_mul(
                        at[ppi][0:64, qqs], osb[0:HD, :], bc[:])
                else:
                    atm = npool.tile([HD, QF], bf16, tag="atm", name="atm", bufs=2)
                    nc.vector.tensor_mul(atm[:], osb[0:HD, :], bc[:])
                    nc.gpsimd.dma_start(out=at[ppi][64:128, qqs], in_=atm[:])

        def flush_norm():
            norm_hops(pending)
            norm_recips(pending)
            norm_bcasts(pending)
            norm_muls(pending)
            pending.clear()

        # ---- prologue: V(heads 0..3), local K0/K1, Q0, Q1 --------------
        # nothing before pi0's attention depends on a collective; the CC
        # engine boots (~25us) + runs the pi>=2 K gathers entirely in
        # the shadow of pi0/pi1's attention. V for heads 4..15 is
        # emitted spread across the first bodies (consumed 2+ pis later).
        for s_ in range(2):
            for tp_ in range(4):
                v_block(s_, tp_, 0, 4)
        k_local(0)
        q_stage(0)
        k_local(1)
        q_stage(1)
        vrest = ([(s_, tp_, 4, 8) for s_ in range(2) for tp_ in range(4)]
                 + [(s_, tp_, 12, 4) for s_ in range(2) for tp_ in range(4)])

        # ---- fused attention + in-stream projection filler -------------
        def qh_block(pi, qh, hook, filler):
            """Emit scores+exp+attn@V for (pi, qh); `hook` (the previous
            block's deferred finisher) runs after kcp0's scores so its
            exp waits hide under fresh matmuls. One `filler` thunk
            (projection chunk / V block) is popped after each odd kcp so
            the exp stream never drains during projection phases.
            Returns this block's own deferred finisher."""
            qt_, kt_ = qts[pi], kts[pi]
            qs = slice(qh * QF, (qh + 1) * QF)
            ope = pso.tile([HD + 1, QF], f32, tag="o", name="o")
            opo = pso.tile([HD + 1, QF], f32, tag="o", name="o")
            pend_e = []

            def attnv_burst(last):
                for bee, beo, bk in pend_e:
                    for j in range(2):
                        kc = 2 * bk + j
                        js = slice(j * QF, (j + 1) * QF)
                        vse = vbig[:, kc // (NT // 2), kc % (NT // 2),
                                   2 * pi, :]
                        vso = vbig[:, kc // (NT // 2), kc % (NT // 2),
                                   2 * pi + 1, :]
                        nc.tensor.matmul(
                            ope[:], vse, bee[:, js],
                            start=(kc == 0), stop=(last and kc == NT - 1))
                        nc.tensor.matmul(
                            opo[:], vso, beo[:, js],
                            start=(kc == 0), stop=(last and kc == NT - 1))
                pend_e.clear()

            for kcp in range(NT // 2):
                spe = u_slot("spe")
                spo = u_slot("spo")
                # even/odd head score MMs adjacent on disjoint PE row
                # groups (h0/h64) and disjoint PSUM banks
                for j in range(2):
                    ks_ = slice((2 * kcp + j) * P, (2 * kcp + j + 1) * P)
                    nc.tensor.matmul(
                        spe[:, j * QF:(j + 1) * QF],
                        kt_[0:64, ks_], qt_[0:64, qs],
                        start=True, stop=True)
                    nc.tensor.matmul(
                        spo[:, j * QF:(j + 1) * QF],
                        kt_[64:128, ks_], qt_[64:128, qs],
                        start=True, stop=True)
                ee = etp.tile([P, SQ], bf16, tag="e", name="e")
                eo = etp.tile([P, SQ], bf16, tag="e", name="e")
                nc.scalar.activation(ee[:], spe[:], AF.Exp, scale=0.125)
                nc.scalar.activation(eo[:], spo[:], AF.Exp, scale=0.125)
                pend_e.append((ee, eo, kcp))
                if kcp == 0 and hook is not None:
                    hook()
                if kcp >= 2 and kcp % 2 == 0:
                    cur = pend_e.pop()
                    attnv_burst(last=False)
                    pend_e.append(cur)
                if kcp % 2 == 1 and filler:
                    th = filler.pop(0)
                    if th is not None:
                        th()

            def finish():
                attnv_burst(last=True)
                for h, op in ((2 * pi, ope), (2 * pi + 1, opo)):
                    osb = npool.tile([HD + 1, QF], f32, tag="osb",
                                     name="osb", bufs=6)
                    nc.vector.tensor_copy(osb[:], op[:])
                    pending.append([h, pi, qs, osb])
            return finish

        def o_qc(qc):
            def run():
                ups = u_slot("ops")
                for oh in range(2):
                    ohs = slice(oh * QF, (oh + 1) * QF)
                    for f in range(ND):
                        nc.tensor.matmul(
                            ups[:, ohs], at[f][:, qc * P:(qc + 1) * P],
                            wo[:, f, ohs],
                            start=(f == 0), stop=(f == ND - 1))
                for oh in range(2):
                    ohs = slice(oh * QF, (oh + 1) * QF)
                    ob = outp.tile([P, QF], bf16, tag="ob", name="ob",
                                   bufs=2)
                    nc.vector.tensor_copy(ob[:], ups[:, ohs])
                    nc.sync.dma_start(out=out[qc * P:(qc + 1) * P, ohs],
                                      in_=ob[:])
            return run

        hook = None
        for pi in range(ND):
            # next-pi K load first: its gather completed a full pi ago
            if 2 <= pi + 1 < ND:
                load_ktile(pi + 1)
            filler = []
            if pi + 2 < ND:
                wsl[pi + 2] = (
                    load_wslice(wqT[:, (pi + 2) * P:(pi + 3) * P], "wq"),
                    load_wslice(wkT[:, (pi + 2) * P:(pi + 3) * P], "wk"))
                filler += k_fill(pi + 2)
                filler += q_fill(pi + 2)
            for _ in range(4):
                if vrest:
                    s_, tp_, h0, nh = vrest.pop(0)
                    filler.append(lambda a=s_, b=tp_, c=h0, d=nh:
                                  v_block(a, b, c, d))
            fin0 = qh_block(pi, 0, hook, filler)

            def hook1(f=fin0):
                f()
                flush_norm()
            fin1 = qh_block(pi, 1, hook1, filler)
            # qh1's finisher is always deferred into the NEXT block's
            # kcp0 (qh0 of pi+1, or o-proj for pi7) for exp runway
            hook = fin1
            # any filler not absorbed by the qh slots lands here
            for th in filler:
                th()
            filler.clear()
            if pi == 5:
                # wo reuses wv's SBUF (tag wbig, bufs=1); emit here so the
                # sync queue has it resident well before o-proj
                wo = wp.tile([P, ND, D], bf16, tag="wbig", name="wo", bufs=1)
                for d_ in range(ND):
                    nc.sync.dma_start(out=wo[:, d_, :],
                                      in_=woT[d_ * P:(d_ + 1) * P, :])

        # ---- o-projection: pi7-qh1's deferred finisher + final norm
        # flush run behind qc0 so qc4..7 find their norms done
        for qc in range(ND):
            if qc == 1 and hook is not None:
                hook()
                flush_norm()
                hook = None
            o_qc(qc)()
    nc.compile()
    return nc


def _rope_tables(pos):
    """pos [n] -> (cos [128, n] bf16, sign-folded TAN [128, n] bf16).

    The kernel computes the sin term as (q*cos) * tan, so the PSUM
    projection tile has a single DVE reader. min |cos| over the table is
    ~6.8e-6 (no bf16 zeros), so q*cos*tan == q*sin to bf16 accuracy."""
    inv = ROPE_BASE ** (-np.arange(0, HD, 2, dtype=np.float64) / HD)
    fr = np.outer(pos.astype(np.float64), inv)          # [n, 32]
    c, s = np.cos(fr), np.sin(fr)
    cos64 = np.concatenate([c, c], axis=1).T            # [64, n]
    tanA = np.concatenate([s / c, -s / c], axis=1).T    # [64, n]
    return (np.tile(cos64, (2, 1)).astype(BF16),
            np.tile(tanA, (2, 1)).astype(BF16))


def _aug_w(w, b):
    """[D, D] weight + [D] bias -> bf16 [D+1, D] (W.T with bias row)."""
    wa = np.empty((D + 1, D), dtype=np.float32)
    wa[:D] = np.asarray(w, dtype=np.float32).T
    wa[D] = np.asarray(b, dtype=np.float32)
    return np.ascontiguousarray(wa).astype(BF16)


def kernel(hidden_states, position_ids, Wq, bq, Wk, bk, Wv, bv, Wo):
    from concourse import bass_utils

    with_bias = bool(
        np.any(np.asarray(bq)) or np.any(np.asarray(bk)) or np.any(np.asarray(bv)))
    key = ("nc", with_bias)
    if key not in _cache:
        _cache[key] = _build_nc(with_bias)
    nc = _cache[key]

    hs = np.asarray(hidden_states, dtype=np.float32)
    pos = np.asarray(position_ids)
    wq = _aug_w(Wq, bq)
    wk = _aug_w(Wk, bk)
    wv = _aug_w(Wv, bv)
    wo = np.ascontiguousarray(np.asarray(Wo, dtype=np.float32).T).astype(BF16)

    in_maps = []
    for core in range(NCORES):
        b, hf = core // 2, core % 2

        def xt_half(h):
            xh = np.empty((D + 1, SQ), dtype=np.float32)
            xh[:D] = hs[b][h * SQ:(h + 1) * SQ].T
            xh[D] = 1.0
            return xh.astype(BF16)

        ck, sk = _rope_tables(np.asarray(pos[b][hf * SQ:(hf + 1) * SQ]))
        cka, ska = _rope_tables(np.asarray(pos[b][0:SQ]))
        ckb, skb = _rope_tables(np.asarray(pos[b][SQ:2 * SQ]))
        in_maps.append({
            "xT": xt_half(hf), "xaT": xt_half(0), "xbT": xt_half(1),
            "wqT": wq, "wkT": wk, "wvT": wv, "woT": wo,
            "cosk": ck, "sink": sk, "cosa": cka, "sina": ska,
            "cosb": ckb, "sinb": skb,
        })

    res = bass_utils.run_bass_kernel_spmd(
        nc, in_maps, core_ids=list(range(NCORES)), trace=TRACE, **TRACE_KW)
    LAST["exec_time_ns"] = res.exec_time_ns
    LAST["mean_exec_time_ns"] = res.mean_exec_time_ns
    LAST["trace"] = res.instructions_and_trace
    LAST["profile_json"] = res.profile_json

    outp_full = np.empty((B, S, D), dtype=np.float32)
    for core in range(NCORES):
        b, hf = core // 2, core % 2
        outp_full[b, hf * SQ:(hf + 1) * SQ] = np.asarray(
            res.results[core]["out"], dtype=np.float32)
    return outp_full


# revision 47
# speedup vs baseline: 1.1676x; 1.1676x over previous
"""Distributed Trainium2 Bass kernel for multi-head attention w/ RoPE.

Reference op (B=4, S=2048, D=1024, H=16, HD=64, fp32):
    q/k/v = hidden @ W{q,k,v}.T + b   (per-head reshape)
    q, k  = rope(q), rope(k)
    out   = softmax(q k^T / sqrt(HD)) v  @ Wo.T

Sharding: 8 cores = 4 batches x 2 query-halves. Each core projects Q
for its own 1024 queries and computes V (and pi0/pi1's K) for BOTH
pair halves locally from xaT/xbT (the pair's x in global slot order --
same data on both pair cores, keeping the SPMD program uniform); K for
pi>=2 is projected for the own half and pairwise-AllGathered. Nothing
before pi0's attention depends on a collective (the CC engine takes
~25us to boot and its early ops run 10-50us). Host unshard is a concat.

Schedule (PE-issue-order is emission order; every stall class found in
the traces has a structural fix here):
  * Unified 3-slot PSUM ring of [128,1024] f32 tiles (6 banks) carries
    every projection / score tile; attn@V accumulators take the last 2
    banks. The ring slack decouples score production from softmax-exp
    (ACT), the steady-state pacer.
  * ACT runs *only* the exp stream (1 elem/cycle/lane is its hard
    floor, ~255us total). RoPE band swaps ride the gpsimd DMA queue;
    all PSUM evictions are DVE; output is stored bf16.
  * Even/odd-head score matmuls issue adjacently on disjoint 64-row PE
    groups and disjoint PSUM banks -> the pair runs CONCURRENTLY in
    the array (measured dt~4ns), halving score time.
  * Projection work is sliced into filler thunks (one 512-token chunk
    + its PSUM-freeing cos-mul) popped between kcp groups INSIDE the
    attention stream, and the remaining V-proj blocks fill later
    bodies, so ACT's exp pipeline never drains at phase boundaries.
  * The last attn@V burst of each query-half (which waits on the final
    exps) is always deferred into the next block's kcp0.
  * RoPE sin term is recovered as (q*cos)*tan -- the tan table is
    sign-folded sin/cos -- so the PSUM tile has a single DVE reader
    and the fp32 read happens once.
All matmuls bf16 with fp32 accumulation; exp folds the 1/sqrt(HD)
scale; the softmax denominator rides an appended ones-column through
attn@V and normalization runs 4-stage (hop/recip/bcast/mul) off the
critical path.
"""

import sys

import numpy as np

try:  # concourse ships in the container; fall back to the staged repo
    import concourse.bass  # noqa: F401
except Exception:  # pragma: no cover
    sys.path.insert(0, "/opt/trn_rl_repo")

import ml_dtypes

B, S, D, H = 4, 2048, 1024, 16
HD = D // H                      # 64
P = 128
NCORES = 8
SQ = S // 2                      # 1024 queries per core
SK = S                           # 2048 keys per core
ND = D // P                      # 8 feature chunks
NT = SK // P                     # 16 key/token chunks
QF = 512                         # matmul moving width
NQF = SQ // QF                   # 2
ROPE_BASE = 10000.0
BF16 = ml_dtypes.bfloat16

TRACE = False                    # test harness flips this
TRACE_KW = {}
LAST = {}                        # exec_time_ns / trace path for test harness

_cache = {}


def _build_nc(with_bias):
    import concourse.bass as bass
    import concourse.mybir as mybir
    import concourse.tile as tile
    from concourse import bacc
    from contextlib import ExitStack

    f32 = mybir.dt.float32
    bf16 = mybir.dt.bfloat16
    AF = mybir.ActivationFunctionType
    PSUM = bass.MemorySpace.PSUM

    nc = bacc.Bacc(None)
    # xT: own query half; xaT/xbT: the PAIR's two halves in global slot
    # order (identical data on both pair cores -- keeps the SPMD program
    # uniform while local K/V match the gathered-K slot order)
    xT = nc.declare_dram_parameter("xT", [D + 1, SQ], bf16, False)
    xaT = nc.declare_dram_parameter("xaT", [D + 1, SQ], bf16, False)
    xbT = nc.declare_dram_parameter("xbT", [D + 1, SQ], bf16, False)
    wqT = nc.declare_dram_parameter("wqT", [D + 1, D], bf16, False)
    wkT = nc.declare_dram_parameter("wkT", [D + 1, D], bf16, False)
    wvT = nc.declare_dram_parameter("wvT", [D + 1, D], bf16, False)
    woT = nc.declare_dram_parameter("woT", [D, D], bf16, False)
    cosk = nc.declare_dram_parameter("cosk", [P, SQ], bf16, False)
    sink = nc.declare_dram_parameter("sink", [P, SQ], bf16, False)
    cosa = nc.declare_dram_parameter("cosa", [P, SQ], bf16, False)
    sina = nc.declare_dram_parameter("sina", [P, SQ], bf16, False)
    cosb = nc.declare_dram_parameter("cosb", [P, SQ], bf16, False)
    sinb = nc.declare_dram_parameter("sinb", [P, SQ], bf16, False)
    out = nc.declare_dram_parameter("out", [SQ, D], bf16, True)
    RG = [[0, 1], [2, 3], [4, 5], [6, 7]]
    BYP = mybir.AluOpType.bypass
    # HBM staging for the pair-wise K AllGathers (V is computed locally
    # from the partner's x half -- no V collectives at all)
    kstg = [nc.dram_tensor(f"kstg{i}", [P, SQ], bf16) for i in range(ND)]
    kgth = [nc.dram_tensor(f"kgth{i}", [2, P, SQ], bf16) for i in range(ND)]
    ccw_in = nc.dram_tensor("ccw_in", [1, 64], bf16)
    ccw_out = nc.dram_tensor("ccw_out", [2, 1, 64], bf16)

    with tile.TileContext(nc) as tc, ExitStack() as st:
        sb = st.enter_context(tc.tile_pool(name="sb", bufs=1))
        qk = st.enter_context(tc.tile_pool(name="qk", bufs=3))
        wp = st.enter_context(tc.tile_pool(name="wp", bufs=2))
        tp = st.enter_context(tc.tile_pool(name="tp", bufs=2))
        etp = st.enter_context(tc.tile_pool(name="et", bufs=5))
        npool = st.enter_context(tc.tile_pool(name="nrm", bufs=3))
        outp = st.enter_context(tc.tile_pool(name="ou", bufs=1))
        psu = st.enter_context(tc.tile_pool(name="psu", bufs=3, space=PSUM))
        pso = st.enter_context(tc.tile_pool(name="pso", bufs=2, space=PSUM))

        def u_slot(nm):
            # one ring slot: [128, 1024] f32 = 2 PSUM banks, 3-deep ring
            return psu.tile([P, SQ], f32, tag="u", name=nm)

        at = [sb.tile([P, SQ], bf16, tag=f"at{i}", name=f"at{i}")
              for i in range(ND)]

        # ---- PE warm-up: dummy matmuls during the initial DMA wait -----
        wu = wp.tile([P, QF], bf16, tag="wu", name="wu", bufs=1)
        nc.vector.memset(wu[:], 0.0)
        psw = u_slot("psw")
        for i in range(14):
            nc.tensor.matmul(psw[:, 0:QF], wu[:, 0:P], wu[:],
                             start=(i == 0), stop=(i == 13))

        # ---- CC warm-up: absorbs the ~30us collective spin-up ----------
        ccwt = wp.tile([1, 64], bf16, tag="ccw", name="ccw", bufs=1)
        nc.vector.memset(ccwt[:], 0.0)
        nc.sync.dma_start(out=ccw_in[:, :], in_=ccwt[:])
        nc.gpsimd.collective_compute(
            "AllGather", BYP, replica_groups=RG,
            ins=[ccw_in[:, :]], outs=[ccw_out[:, :, :]])

        # ---- loads (first K/Q weight slices first so K proj can start
        # streaming behind the x chunks as they land) -------------------
        def load_wslice(wdram, wtag):
            ws = wp.tile([P, ND, P], bf16, tag=wtag, name=wtag)
            nc.sync.dma_start(
                out=ws[:],
                in_=wdram[0:D, :].rearrange("(n p) o -> p n o", p=P))
            wb = None
            if with_bias:
                wb = wp.tile([1, P], bf16, tag=wtag + "b", name=wtag + "b")
                nc.sync.dma_start(out=wb[:], in_=wdram[D:D + 1, :])
            return ws, wb

        # V proj (first in PE order) contracts wv x (xa,xb): interleave
        # those DMAs so its d-chunk matmuls stream behind the arrivals;
        # xs (only needed by Q proj, later) loads after.
        wv = wp.tile([P, ND, D], bf16, tag="wbig", name="wv", bufs=1)
        xs = [sb.tile([P, SQ], bf16, tag=f"x{d}", name=f"x{d}")
              for d in range(ND)]
        xa = [sb.tile([P, SQ], bf16, tag=f"xa{d}", name=f"xa{d}")
              for d in range(ND)]
        xb = [sb.tile([P, SQ], bf16, tag=f"xb{d}", name=f"xb{d}")
              for d in range(ND)]
        for d_ in range(ND):
            nc.sync.dma_start(out=wv[:, d_, 0:4 * HD],
                              in_=wvT[d_ * P:(d_ + 1) * P, 0:4 * HD])
            nc.sync.dma_start(out=xa[d_][:], in_=xaT[d_ * P:(d_ + 1) * P, :])
            nc.sync.dma_start(out=xb[d_][:], in_=xbT[d_ * P:(d_ + 1) * P, :])
        cks = {}
        for nm, src in (("ck", cosk), ("sk", sink), ("cka", cosa),
                        ("ska", sina), ("ckb", cosb), ("skb", sinb)):
            cks[nm] = sb.tile([P, SQ], bf16, tag=nm, name=nm)
            nc.sync.dma_start(out=cks[nm][:], in_=src[:, :])
        ck, sk_ = cks["ck"], cks["sk"]
        cka, ska, ckb, skb = cks["cka"], cks["ska"], cks["ckb"], cks["skb"]
        wsl = {0: (load_wslice(wqT[:, 0:P], "wq"), load_wslice(wkT[:, 0:P], "wk")),
               1: (load_wslice(wqT[:, P:2 * P], "wq"),
                   load_wslice(wkT[:, P:2 * P], "wk"))}
        for d_ in range(ND):
            nc.sync.dma_start(out=xs[d_][:], in_=xT[d_ * P:(d_ + 1) * P, :])
        for d_ in range(ND):
            nc.sync.dma_start(out=wv[:, d_, 4 * HD:D],
                              in_=wvT[d_ * P:(d_ + 1) * P, 4 * HD:D])
        xone = xaone = xbone = None
        if with_bias:
            xone = sb.tile([1, SQ], bf16, tag="xone", name="xone")
            nc.sync.dma_start(out=xone[:], in_=xT[D:D + 1, :])
            xaone = sb.tile([1, SQ], bf16, tag="xaone", name="xaone")
            nc.sync.dma_start(out=xaone[:], in_=xaT[D:D + 1, :])
            xbone = sb.tile([1, SQ], bf16, tag="xbone", name="xbone")
            nc.sync.dma_start(out=xbone[:], in_=xbT[D:D + 1, :])
            wvb = wp.tile([1, D], bf16, tag="wvb", name="wvb", bufs=1)
            nc.sync.dma_start(out=wvb[:], in_=wvT[D:D + 1, :])

        def qk_proj(wsb, dst, xv, ctbl, stbl, xo):
            """dst[o128, 0:SQ] = rope(W[pi-slice] @ xv^T + b). The cos mul
            (the only PSUM reader) frees the u-slot after ONE DVE op; the
            sin term is recovered as dst * tan (stbl holds the
            sign-folded sin/cos table), an all-SBUF bf16 mul at the DVE
            fast rate. The 2-pi pipeline hides all the swap latency."""
            ws, wb = wsb
            ups = u_slot("ups")
            for c in range(NQF):
                cs = slice(c * QF, (c + 1) * QF)
                for d_ in range(ND):
                    nc.tensor.matmul(
                        ups[:, cs], ws[:, d_, :], xv[d_][:, cs],
                        start=(d_ == 0), stop=(not with_bias and d_ == ND - 1))
                if with_bias:
                    nc.tensor.matmul(
                        ups[:, cs], wb[:], xo[:, cs],
                        start=False, stop=True)
            t2 = tp.tile([P, SQ], bf16, tag="t2", name="t2")
            t2s = tp.tile([P, SQ], bf16, tag="t2s", name="t2s")
            nc.vector.tensor_mul(dst, ups[:], ctbl[:])
            nc.vector.tensor_mul(t2[:], dst, stbl[:])
            for b0 in (0, 64):
                nc.gpsimd.dma_start(out=t2s[b0:b0 + 32, :],
                                    in_=t2[b0 + 32:b0 + 64, :])
                nc.gpsimd.dma_start(out=t2s[b0 + 32:b0 + 64, :],
                                    in_=t2[b0:b0 + 32, :])
            nc.vector.tensor_add(dst, dst, t2s[:])

        kts, qts, vps = {}, {}, {}

        def qk_chunks(wsb, dtile, dbase, xv, ctbl, stbl, xo, post=None):
            """Split projection: two filler thunks, one per 512-token
            chunk. Each allocates its own (half-used) u-slot, freed by
            its cos-mul; the second finishes rope (tan mul + band swap +
            add) and runs `post` (K staging). Emitted INSIDE the
            attention stream so ACT's exp pipeline never drains."""
            ws, wb = wsb

            def chunk(c, fin):
                def run():
                    ups = u_slot("ups")
                    cs = slice(c * QF, (c + 1) * QF)
                    for d_ in range(ND):
                        nc.tensor.matmul(
                            ups[:, 0:QF], ws[:, d_, :], xv[d_][:, cs],
                            start=(d_ == 0),
                            stop=(not with_bias and d_ == ND - 1))
                    if with_bias:
                        nc.tensor.matmul(
                            ups[:, 0:QF], wb[:], xo[:, cs],
                            start=False, stop=True)
                    dsl = dtile[:, dbase + c * QF:dbase + (c + 1) * QF]
                    nc.vector.tensor_mul(dsl, ups[:, 0:QF], ctbl[:, cs])
                    if fin:
                        dst = dtile[:, dbase:dbase + SQ]
                        t2 = tp.tile([P, SQ], bf16, tag="t2", name="t2")
                        t2s = tp.tile([P, SQ], bf16, tag="t2s", name="t2s")
                        nc.vector.tensor_mul(t2[:], dst, stbl[:])
                        for b0 in (0, 64):
                            nc.gpsimd.dma_start(out=t2s[b0:b0 + 32, :],
                                                in_=t2[b0 + 32:b0 + 64, :])
                            nc.gpsimd.dma_start(out=t2s[b0 + 32:b0 + 64, :],
                                                in_=t2[b0:b0 + 32, :])
                        nc.vector.tensor_add(dst, dst, t2s[:])
                        if post is not None:
                            post()
                return run
            return [chunk(0, False), chunk(1, True)]

        def k_fill(pi_):
            kt_ = qk.tile([P, SK], bf16, tag="kt", name="kt", bufs=3)
            kts[pi_] = kt_

            def post():
                nc.sync.dma_start(out=kstg[pi_][:, :], in_=kt_[:, 0:SQ])
                nc.gpsimd.collective_compute(
                    "AllGather", BYP, replica_groups=RG,
                    ins=[kstg[pi_][:, :]], outs=[kgth[pi_][:, :, :]])
            return qk_chunks(wsl[pi_][1], kt_, 0, xs, ck, sk_, xone, post)

        def q_fill(pi_):
            qt_ = qk.tile([P, SQ], bf16, tag="qt", name="qt", bufs=3)
            qts[pi_] = qt_
            return qk_chunks(wsl[pi_][0], qt_, 0, xs, ck, sk_, xone)

        def k_local(pi_):
            # pi0/pi1: project BOTH pair-halves locally in slot order --
            # no collective gates the startup
            kt_ = qk.tile([P, SK], bf16, tag="kt", name="kt", bufs=3)
            qk_proj(wsl[pi_][1], kt_[:, 0:SQ], xa, cka, ska, xaone)
            qk_proj(wsl[pi_][1], kt_[:, SQ:SK], xb, ckb, skb, xbone)
            kts[pi_] = kt_

        def q_stage(pi_):
            qt_ = qk.tile([P, SQ], bf16, tag="qt", name="qt", bufs=3)
            qk_proj(wsl[pi_][0], qt_[:], xs, ck, sk_, xone)
            qts[pi_] = qt_

        def load_ktile(pi_):
            for s_ in range(2):
                nc.sync.dma_start(out=kts[pi_][:, s_ * SQ:(s_ + 1) * SQ],
                                  in_=kgth[pi_][s_, :, :])

        # V for ALL heads and BOTH pair-halves lives in one big SBUF
        # tile, computed locally from xa/xb in global slot order (so it
        # agrees with both the local pi0/pi1 K tiles and the gathered-K
        # slot order). No V collectives exist at all.
        vbig = sb.tile([P, 2, NT // 2, H, HD + 1], bf16, tag="vbig",
                       name="vbig")
        nc.vector.memset(vbig[:, :, :, :, HD:HD + 1], 1.0)

        def v_block(s_, tp_, h0, nh):
            """V proj for heads [h0, h0+nh) x token chunks (2tp_, 2tp_+1)
            of pair-half s_: one u-slot, two nh*64-wide accum groups."""
            xv, xo = (xa, xaone) if s_ == 0 else (xb, xbone)
            hw = nh * HD
            ups = u_slot("vps")
            for ti in range(2):
                t_ = 2 * tp_ + ti
                tqs = slice(ti * hw, (ti + 1) * hw)
                ohs = slice(h0 * HD, h0 * HD + hw)
                for d_ in range(ND):
                    nc.tensor.matmul(
                        ups[:, tqs], xv[d_][:, t_ * P:(t_ + 1) * P],
                        wv[:, d_, ohs],
                        start=(d_ == 0),
                        stop=(not with_bias and d_ == ND - 1))
                if with_bias:
                    nc.tensor.matmul(
                        ups[:, tqs], xo[:, t_ * P:(t_ + 1) * P],
                        wvb[:, ohs], start=False, stop=True)
            nc.vector.tensor_copy(
                vbig[:, s_, 2 * tp_:2 * tp_ + 2, h0:h0 + nh, 0:HD],
                ups[:, 0:2 * hw].rearrange("p (t h d) -> p t h d",
                                           h=nh, d=HD))

        # ---- normalization stage machinery (off the critical path) ----
        pending = []

        def norm_hops(batch):
            for ent in batch:
                den = npool.tile([1, QF], f32, tag="den", name="den", bufs=4)
                nc.gpsimd.dma_start(out=den[:], in_=ent[3][HD:HD + 1, :])
                ent.append(den)

        def norm_recips(batch):
            for ent in batch:
                rc = npool.tile([1, QF], f32, tag="rc", name="rc", bufs=4)
                nc.vector.reciprocal_approx_fast(rc[:], ent[4][:])
                ent.append(rc)

        def norm_bcasts(batch):
            for ent in batch:
                bc = npool.tile([HD, QF], f32, tag="bc", name="bc", bufs=3)
                nc.gpsimd.partition_broadcast(bc[:], ent[5][:])
                ent.append(bc)

        def norm_muls(batch):
            for h, ppi, qqs, osb, den, rc, bc in batch:
                if h % 2 == 0:
                    nc.vector.tensor_mul(
                        at[ppi][0:64, qqs], osb[0:HD, :], bc[:])
                else:
                    atm = npool.tile([HD, QF], bf16, tag="atm", name="atm", bufs=2)
                    nc.vector.tensor_mul(atm[:], osb[0:HD, :], bc[:])
                    nc.gpsimd.dma_start(out=at[ppi][64:128, qqs], in_=atm[:])

        def flush_norm():
            norm_hops(pending)
            norm_recips(pending)
            norm_bcasts(pending)
            norm_muls(pending)
            pending.clear()

        # ---- prologue: V(heads 0..3), local K0/K1, Q0, Q1 --------------
        # nothing before pi0's attention depends on a collective; the CC
        # engine boots (~25us) + runs the pi>=2 K gathers entirely in
        # the shadow of pi0/pi1's attention. V for heads 4..15 is
        # emitted spread across the first bodies (consumed 2+ pis later).
        for s_ in range(2):
            for tp_ in range(4):
                v_block(s_, tp_, 0, 4)
        k_local(0)
        q_stage(0)
        k_local(1)
        q_stage(1)
        vrest = [(s_, tp_, h0, 4) for h0 in (4, 8, 12)
                 for s_ in range(2) for tp_ in range(4)]

        # ---- fused attention + in-stream projection filler -------------
        def qh_block(pi, qh, hook, filler):
            """Emit scores+exp+attn@V for (pi, qh); `hook` (the previous
            block's deferred finisher) runs after kcp0's scores so its
            exp waits hide under fresh matmuls. One `filler` thunk
            (projection chunk / V block) is popped after each odd kcp so
            the exp stream never drains during projection phases.
            Returns this block's own deferred finisher."""
            qt_, kt_ = qts[pi], kts[pi]
            qs = slice(qh * QF, (qh + 1) * QF)
            ope = pso.tile([HD + 1, QF], f32, tag="o", name="o")
            opo = pso.tile([HD + 1, QF], f32, tag="o", name="o")
            pend_e = []

            def attnv_burst(last):
                for bee, beo, bk in pend_e:
                    for j in range(2):
                        kc = 2 * bk + j
                        js = slice(j * QF, (j + 1) * QF)
                        vse = vbig[:, kc // (NT // 2), kc % (NT // 2),
                                   2 * pi, :]
                        vso = vbig[:, kc // (NT // 2), kc % (NT // 2),
                                   2 * pi + 1, :]
                        nc.tensor.matmul(
                            ope[:], vse, bee[:, js],
                            start=(kc == 0), stop=(last and kc == NT - 1))
                        nc.tensor.matmul(
                            opo[:], vso, beo[:, js],
                            start=(kc == 0), stop=(last and kc == NT - 1))
                pend_e.clear()

            for kcp in range(NT // 2):
                spe = u_slot("spe")
                spo = u_slot("spo")
                # even/odd head score MMs adjacent on disjoint PE row
                # groups (h0/h64) and disjoint PSUM banks
                for j in range(2):
                    ks_ = slice((2 * kcp + j) * P, (2 * kcp + j + 1) * P)
                    nc.tensor.matmul(
                        spe[:, j * QF:(j + 1) * QF],
                        kt_[0:64, ks_], qt_[0:64, qs],
                        start=True, stop=True)
                    nc.tensor.matmul(
                        spo[:, j * QF:(j + 1) * QF],
                        kt_[64:128, ks_], qt_[64:128, qs],
                        start=True, stop=True)
                ee = etp.tile([P, SQ], bf16, tag="e", name="e")
                eo = etp.tile([P, SQ], bf16, tag="e", name="e")
                nc.scalar.activation(ee[:], spe[:], AF.Exp, scale=0.125)
                nc.scalar.activation(eo[:], spo[:], AF.Exp, scale=0.125)
                pend_e.append((ee, eo, kcp))
                if kcp == 0 and hook is not None:
                    hook()
                if kcp >= 2 and kcp % 2 == 0:
                    cur = pend_e.pop()
                    attnv_burst(last=False)
                    pend_e.append(cur)
                if kcp % 2 == 1 and filler:
                    th = filler.pop(0)
                    if th is not None:
                        th()

            def finish():
                attnv_burst(last=True)
                for h, op in ((2 * pi, ope), (2 * pi + 1, opo)):
                    osb = npool.tile([HD + 1, QF], f32, tag="osb",
                                     name="osb", bufs=6)
                    nc.vector.tensor_copy(osb[:], op[:])
                    pending.append([h, pi, qs, osb])
            return finish

        def o_qc(qc):
            def run():
                ups = u_slot("ops")
                for oh in range(2):
                    ohs = slice(oh * QF, (oh + 1) * QF)
                    for f in range(ND):
                        nc.tensor.matmul(
                            ups[:, ohs], at[f][:, qc * P:(qc + 1) * P],
                            wo[:, f, ohs],
                            start=(f == 0), stop=(f == ND - 1))
                for oh in range(2):
                    ohs = slice(oh * QF, (oh + 1) * QF)
                    ob = outp.tile([P, QF], bf16, tag="ob", name="ob",
                                   bufs=2)
                    nc.vector.tensor_copy(ob[:], ups[:, ohs])
                    nc.sync.dma_start(out=out[qc * P:(qc + 1) * P, ohs],
                                      in_=ob[:])
            return run

        hook = None
        for pi in range(ND):
            # next-pi K load first: its gather completed a full pi ago
            if 2 <= pi + 1 < ND:
                load_ktile(pi + 1)
            filler = []
            if pi + 2 < ND:
                wsl[pi + 2] = (
                    load_wslice(wqT[:, (pi + 2) * P:(pi + 3) * P], "wq"),
                    load_wslice(wkT[:, (pi + 2) * P:(pi + 3) * P], "wk"))
                filler += k_fill(pi + 2)
                filler += q_fill(pi + 2)
            for _ in range(4):
                if vrest:
                    s_, tp_, h0, nh = vrest.pop(0)
                    filler.append(lambda a=s_, b=tp_, c=h0, d=nh:
                                  v_block(a, b, c, d))
            fin0 = qh_block(pi, 0, hook, filler)

            def hook1(f=fin0):
                f()
                flush_norm()
            fin1 = qh_block(pi, 1, hook1, filler)
            # qh1's finisher is always deferred into the NEXT block's
            # kcp0 (qh0 of pi+1, or o-proj for pi7) for exp runway
            hook = fin1
            # any filler not absorbed by the qh slots lands here
            for th in filler:
                th()
            filler.clear()
            if pi == 5:
                # wo reuses wv's SBUF (tag wbig, bufs=1); emit here so the
                # sync queue has it resident well before o-proj
                wo = wp.tile([P, ND, D], bf16, tag="wbig", name="wo", bufs=1)
                for d_ in range(ND):
                    nc.sync.dma_start(out=wo[:, d_, :],
                                      in_=woT[d_ * P:(d_ + 1) * P, :])

        # ---- o-projection: pi7-qh1's deferred finisher + final norm
        # flush run behind qc0 so qc4..7 find their norms done
        for qc in range(ND):
            if qc == 1 and hook is not None:
                hook()
                flush_norm()
                hook = None
            o_qc(qc)()
    nc.compile()
    return nc


def _rope_tables(pos):
    """pos [n] -> (cos [128, n] bf16, sign-folded TAN [128, n] bf16).

    The kernel computes the sin term as (q*cos) * tan, so the PSUM
    projection tile has a single DVE reader. min |cos| over the table is
    ~6.8e-6 (no bf16 zeros), so q*cos*tan == q*sin to bf16 accuracy."""
    inv = ROPE_BASE ** (-np.arange(0, HD, 2, dtype=np.float64) / HD)
    fr = np.outer(pos.astype(np.float64), inv)          # [n, 32]
    c, s = np.cos(fr), np.sin(fr)
    cos64 = np.concatenate([c, c], axis=1).T            # [64, n]
    tanA = np.concatenate([s / c, -s / c], axis=1).T    # [64, n]
    return (np.tile(cos64, (2, 1)).astype(BF16),
            np.tile(tanA, (2, 1)).astype(BF16))


def _aug_w(w, b):
    """[D, D] weight + [D] bias -> bf16 [D+1, D] (W.T with bias row)."""
    wa = np.empty((D + 1, D), dtype=np.float32)
    wa[:D] = np.asarray(w, dtype=np.float32).T
    wa[D] = np.asarray(b, dtype=np.float32)
    return np.ascontiguousarray(wa).astype(BF16)


def kernel(hidden_states, position_ids, Wq, bq, Wk, bk, Wv, bv, Wo):
    from concourse import bass_utils

    with_bias = bool(
        np.any(np.asarray(bq)) or np.any(np.asarray(bk)) or np.any(np.asarray(bv)))
    key = ("nc", with_bias)
    if key not in _cache:
        _cache[key] = _build_nc(with_bias)
    nc = _cache[key]

    hs = np.asarray(hidden_states, dtype=np.float32)
    pos = np.asarray(position_ids)
    wq = _aug_w(Wq, bq)
    wk = _aug_w(Wk, bk)
    wv = _aug_w(Wv, bv)
    wo = np.ascontiguousarray(np.asarray(Wo, dtype=np.float32).T).astype(BF16)

    in_maps = []
    for core in range(NCORES):
        b, hf = core // 2, core % 2

        def xt_half(h):
            xh = np.empty((D + 1, SQ), dtype=np.float32)
            xh[:D] = hs[b][h * SQ:(h + 1) * SQ].T
            xh[D] = 1.0
            return xh.astype(BF16)

        ck, sk = _rope_tables(np.asarray(pos[b][hf * SQ:(hf + 1) * SQ]))
        cka, ska = _rope_tables(np.asarray(pos[b][0:SQ]))
        ckb, skb = _rope_tables(np.asarray(pos[b][SQ:2 * SQ]))
        in_maps.append({
            "xT": xt_half(hf), "xaT": xt_half(0), "xbT": xt_half(1),
            "wqT": wq, "wkT": wk, "wvT": wv, "woT": wo,
            "cosk": ck, "sink": sk, "cosa": cka, "sina": ska,
            "cosb": ckb, "sinb": skb,
        })

    res = bass_utils.run_bass_kernel_spmd(
        nc, in_maps, core_ids=list(range(NCORES)), trace=TRACE, **TRACE_KW)
    LAST["exec_time_ns"] = res.exec_time_ns
    LAST["mean_exec_time_ns"] = res.mean_exec_time_ns
    LAST["trace"] = res.instructions_and_trace
    LAST["profile_json"] = res.profile_json

    outp_full = np.empty((B, S, D), dtype=np.float32)
    for core in range(NCORES):
        b, hf = core // 2, core % 2
        outp_full[b, hf * SQ:(hf + 1) * SQ] = np.asarray(
            res.results[core]["out"], dtype=np.float32)
    return outp_full


# revision 48
# speedup vs baseline: 1.1769x; 1.0080x over previous
"""Distributed Trainium2 Bass kernel for multi-head attention w/ RoPE.

Reference op (B=4, S=2048, D=1024, H=16, HD=64, fp32):
    q/k/v = hidden @ W{q,k,v}.T + b   (per-head reshape)
    q, k  = rope(q), rope(k)
    out   = softmax(q k^T / sqrt(HD)) v  @ Wo.T

Sharding: 8 cores = 4 batches x 2 query-halves. Each core projects Q
for its own 1024 queries and computes V (and pi0/pi1's K) for BOTH
pair halves locally from xaT/xbT (the pair's x in global slot order --
same data on both pair cores, keeping the SPMD program uniform); K for
pi>=2 is projected for the own half and pairwise-AllGathered. Nothing
before pi0's attention depends on a collective (the CC engine takes
~25us to boot and its early ops run 10-50us). Host unshard is a concat.

Schedule (PE-issue-order is emission order; every stall class found in
the traces has a structural fix here):
  * Unified 3-slot PSUM ring of [128,1024] f32 tiles (6 banks) carries
    every projection / score tile; attn@V accumulators take the last 2
    banks. The ring slack decouples score production from softmax-exp
    (ACT), the steady-state pacer.
  * ACT runs *only* the exp stream (1 elem/cycle/lane is its hard
    floor, ~255us total). RoPE band swaps ride the gpsimd DMA queue;
    all PSUM evictions are DVE; output is stored bf16.
  * Even/odd-head score matmuls issue adjacently on disjoint 64-row PE
    groups and disjoint PSUM banks -> the pair runs CONCURRENTLY in
    the array (measured dt~4ns), halving score time.
  * Projection work is sliced into filler thunks (one 512-token chunk
    + its PSUM-freeing cos-mul) popped between kcp groups INSIDE the
    attention stream, and the remaining V-proj blocks fill later
    bodies, so ACT's exp pipeline never drains at phase boundaries.
  * The last attn@V burst of each query-half (which waits on the final
    exps) is always deferred into the next block's kcp0.
  * RoPE sin term is recovered as (q*cos)*tan -- the tan table is
    sign-folded sin/cos -- so the PSUM tile has a single DVE reader
    and the fp32 read happens once.
All matmuls bf16 with fp32 accumulation; exp folds the 1/sqrt(HD)
scale; the softmax denominator rides an appended ones-column through
attn@V and normalization runs 4-stage (hop/recip/bcast/mul) off the
critical path.
"""

import sys

import numpy as np

try:  # concourse ships in the container; fall back to the staged repo
    import concourse.bass  # noqa: F401
except Exception:  # pragma: no cover
    sys.path.insert(0, "/opt/trn_rl_repo")

import ml_dtypes

B, S, D, H = 4, 2048, 1024, 16
HD = D // H                      # 64
P = 128
NCORES = 8
SQ = S // 2                      # 1024 queries per core
SK = S                           # 2048 keys per core
ND = D // P                      # 8 feature chunks
NT = SK // P                     # 16 key/token chunks
QF = 512                         # matmul moving width
NQF = SQ // QF                   # 2
ROPE_BASE = 10000.0
BF16 = ml_dtypes.bfloat16

TRACE = False                    # test harness flips this
TRACE_KW = {}
LAST = {}                        # exec_time_ns / trace path for test harness

_cache = {}


def _build_nc(with_bias):
    import concourse.bass as bass
    import concourse.mybir as mybir
    import concourse.tile as tile
    from concourse import bacc
    from contextlib import ExitStack

    f32 = mybir.dt.float32
    bf16 = mybir.dt.bfloat16
    AF = mybir.ActivationFunctionType
    PSUM = bass.MemorySpace.PSUM

    nc = bacc.Bacc(None)
    # xT: own query half; xaT/xbT: the PAIR's two halves in global slot
    # order (identical data on both pair cores -- keeps the SPMD program
    # uniform while local K/V match the gathered-K slot order)
    xT = nc.declare_dram_parameter("xT", [D + 1, SQ], bf16, False)
    xaT = nc.declare_dram_parameter("xaT", [D + 1, SQ], bf16, False)
    xbT = nc.declare_dram_parameter("xbT", [D + 1, SQ], bf16, False)
    wqT = nc.declare_dram_parameter("wqT", [D + 1, D], bf16, False)
    wkT = nc.declare_dram_parameter("wkT", [D + 1, D], bf16, False)
    wvT = nc.declare_dram_parameter("wvT", [D + 1, D], bf16, False)
    woT = nc.declare_dram_parameter("woT", [D, D], bf16, False)
    cosk = nc.declare_dram_parameter("cosk", [P, SQ], bf16, False)
    sink = nc.declare_dram_parameter("sink", [P, SQ], bf16, False)
    cosa = nc.declare_dram_parameter("cosa", [P, SQ], bf16, False)
    sina = nc.declare_dram_parameter("sina", [P, SQ], bf16, False)
    cosb = nc.declare_dram_parameter("cosb", [P, SQ], bf16, False)
    sinb = nc.declare_dram_parameter("sinb", [P, SQ], bf16, False)
    out = nc.declare_dram_parameter("out", [SQ, D], bf16, True)
    RG = [[0, 1], [2, 3], [4, 5], [6, 7]]
    BYP = mybir.AluOpType.bypass
    # HBM staging for the pair-wise K AllGathers (V is computed locally
    # from the partner's x half -- no V collectives at all)
    kstg = [nc.dram_tensor(f"kstg{i}", [P, SQ], bf16) for i in range(ND)]
    kgth = [nc.dram_tensor(f"kgth{i}", [2, P, SQ], bf16) for i in range(ND)]
    ccw_in = nc.dram_tensor("ccw_in", [1, 64], bf16)
    ccw_out = nc.dram_tensor("ccw_out", [2, 1, 64], bf16)

    with tile.TileContext(nc) as tc, ExitStack() as st:
        sb = st.enter_context(tc.tile_pool(name="sb", bufs=1))
        qk = st.enter_context(tc.tile_pool(name="qk", bufs=3))
        wp = st.enter_context(tc.tile_pool(name="wp", bufs=2))
        tp = st.enter_context(tc.tile_pool(name="tp", bufs=2))
        etp = st.enter_context(tc.tile_pool(name="et", bufs=5))
        npool = st.enter_context(tc.tile_pool(name="nrm", bufs=3))
        outp = st.enter_context(tc.tile_pool(name="ou", bufs=1))
        psu = st.enter_context(tc.tile_pool(name="psu", bufs=3, space=PSUM))
        pso = st.enter_context(tc.tile_pool(name="pso", bufs=2, space=PSUM))

        def u_slot(nm):
            # one ring slot: [128, 1024] f32 = 2 PSUM banks, 3-deep ring
            return psu.tile([P, SQ], f32, tag="u", name=nm)

        at = [sb.tile([P, SQ], bf16, tag=f"at{i}", name=f"at{i}")
              for i in range(ND)]

        # ---- PE warm-up: dummy matmuls during the initial DMA wait -----
        wu = wp.tile([P, QF], bf16, tag="wu", name="wu", bufs=1)
        nc.vector.memset(wu[:], 0.0)
        psw = u_slot("psw")
        for i in range(14):
            nc.tensor.matmul(psw[:, 0:QF], wu[:, 0:P], wu[:],
                             start=(i == 0), stop=(i == 13))

        # ---- CC warm-up: absorbs the ~30us collective spin-up ----------
        ccwt = wp.tile([1, 64], bf16, tag="ccw", name="ccw", bufs=1)
        nc.vector.memset(ccwt[:], 0.0)
        nc.sync.dma_start(out=ccw_in[:, :], in_=ccwt[:])
        nc.gpsimd.collective_compute(
            "AllGather", BYP, replica_groups=RG,
            ins=[ccw_in[:, :]], outs=[ccw_out[:, :, :]])

        # ---- loads (first K/Q weight slices first so K proj can start
        # streaming behind the x chunks as they land) -------------------
        def load_wslice(wdram, wtag):
            ws = wp.tile([P, ND, P], bf16, tag=wtag, name=wtag)
            nc.sync.dma_start(
                out=ws[:],
                in_=wdram[0:D, :].rearrange("(n p) o -> p n o", p=P))
            wb = None
            if with_bias:
                wb = wp.tile([1, P], bf16, tag=wtag + "b", name=wtag + "b")
                nc.sync.dma_start(out=wb[:], in_=wdram[D:D + 1, :])
            return ws, wb

        # V proj (first in PE order) contracts wv x (xa,xb): interleave
        # those DMAs so its d-chunk matmuls stream behind the arrivals;
        # xs (only needed by Q proj, later) loads after.
        wv = wp.tile([P, ND, D], bf16, tag="wbig", name="wv", bufs=1)
        xs = [sb.tile([P, SQ], bf16, tag=f"x{d}", name=f"x{d}")
              for d in range(ND)]
        xa = [sb.tile([P, SQ], bf16, tag=f"xa{d}", name=f"xa{d}")
              for d in range(ND)]
        xb = [sb.tile([P, SQ], bf16, tag=f"xb{d}", name=f"xb{d}")
              for d in range(ND)]
        for d_ in range(ND):
            nc.sync.dma_start(out=wv[:, d_, 0:4 * HD],
                              in_=wvT[d_ * P:(d_ + 1) * P, 0:4 * HD])
            nc.sync.dma_start(out=xa[d_][:], in_=xaT[d_ * P:(d_ + 1) * P, :])
            nc.sync.dma_start(out=xb[d_][:], in_=xbT[d_ * P:(d_ + 1) * P, :])
        cks = {}
        for nm, src in (("ck", cosk), ("sk", sink), ("cka", cosa),
                        ("ska", sina), ("ckb", cosb), ("skb", sinb)):
            cks[nm] = sb.tile([P, SQ], bf16, tag=nm, name=nm)
            nc.sync.dma_start(out=cks[nm][:], in_=src[:, :])
        ck, sk_ = cks["ck"], cks["sk"]
        cka, ska, ckb, skb = cks["cka"], cks["ska"], cks["ckb"], cks["skb"]
        wsl = {0: (load_wslice(wqT[:, 0:P], "wq"), load_wslice(wkT[:, 0:P], "wk")),
               1: (load_wslice(wqT[:, P:2 * P], "wq"),
                   load_wslice(wkT[:, P:2 * P], "wk"))}
        for d_ in range(ND):
            nc.sync.dma_start(out=xs[d_][:], in_=xT[d_ * P:(d_ + 1) * P, :])
        for d_ in range(ND):
            nc.sync.dma_start(out=wv[:, d_, 4 * HD:D],
                              in_=wvT[d_ * P:(d_ + 1) * P, 4 * HD:D])
        xone = xaone = xbone = None
        if with_bias:
            xone = sb.tile([1, SQ], bf16, tag="xone", name="xone")
            nc.sync.dma_start(out=xone[:], in_=xT[D:D + 1, :])
            xaone = sb.tile([1, SQ], bf16, tag="xaone", name="xaone")
            nc.sync.dma_start(out=xaone[:], in_=xaT[D:D + 1, :])
            xbone = sb.tile([1, SQ], bf16, tag="xbone", name="xbone")
            nc.sync.dma_start(out=xbone[:], in_=xbT[D:D + 1, :])
            wvb = wp.tile([1, D], bf16, tag="wvb", name="wvb", bufs=1)
            nc.sync.dma_start(out=wvb[:], in_=wvT[D:D + 1, :])

        def qk_proj(wsb, dst, xv, ctbl, stbl, xo):
            """dst[o128, 0:SQ] = rope(W[pi-slice] @ xv^T + b). The cos mul
            (the only PSUM reader) frees the u-slot after ONE DVE op; the
            sin term is recovered as dst * tan (stbl holds the
            sign-folded sin/cos table), an all-SBUF bf16 mul at the DVE
            fast rate. The 2-pi pipeline hides all the swap latency."""
            ws, wb = wsb
            ups = u_slot("ups")
            for c in range(NQF):
                cs = slice(c * QF, (c + 1) * QF)
                for d_ in range(ND):
                    nc.tensor.matmul(
                        ups[:, cs], ws[:, d_, :], xv[d_][:, cs],
                        start=(d_ == 0), stop=(not with_bias and d_ == ND - 1))
                if with_bias:
                    nc.tensor.matmul(
                        ups[:, cs], wb[:], xo[:, cs],
                        start=False, stop=True)
            t2 = tp.tile([P, SQ], bf16, tag="t2", name="t2")
            t2s = tp.tile([P, SQ], bf16, tag="t2s", name="t2s")
            nc.vector.tensor_mul(dst, ups[:], ctbl[:])
            nc.vector.tensor_mul(t2[:], dst, stbl[:])
            for b0 in (0, 64):
                nc.gpsimd.dma_start(out=t2s[b0:b0 + 32, :],
                                    in_=t2[b0 + 32:b0 + 64, :])
                nc.gpsimd.dma_start(out=t2s[b0 + 32:b0 + 64, :],
                                    in_=t2[b0:b0 + 32, :])
            nc.vector.tensor_add(dst, dst, t2s[:])

        kts, qts, vps = {}, {}, {}

        def qk_chunks(wsb, dtile, dbase, xv, ctbl, stbl, xo, post=None):
            """Split projection: two filler thunks, one per 512-token
            chunk. Each allocates its own (half-used) u-slot, freed by
            its cos-mul; the second finishes rope (tan mul + band swap +
            add) and runs `post` (K staging). Emitted INSIDE the
            attention stream so ACT's exp pipeline never drains."""
            ws, wb = wsb

            def chunk(c, fin):
                def run():
                    ups = u_slot("ups")
                    cs = slice(c * QF, (c + 1) * QF)
                    for d_ in range(ND):
                        nc.tensor.matmul(
                            ups[:, 0:QF], ws[:, d_, :], xv[d_][:, cs],
                            start=(d_ == 0),
                            stop=(not with_bias and d_ == ND - 1))
                    if with_bias:
                        nc.tensor.matmul(
                            ups[:, 0:QF], wb[:], xo[:, cs],
                            start=False, stop=True)
                    dsl = dtile[:, dbase + c * QF:dbase + (c + 1) * QF]
                    nc.vector.tensor_mul(dsl, ups[:, 0:QF], ctbl[:, cs])
                    if fin:
                        dst = dtile[:, dbase:dbase + SQ]
                        t2 = tp.tile([P, SQ], bf16, tag="t2", name="t2")
                        t2s = tp.tile([P, SQ], bf16, tag="t2s", name="t2s")
                        nc.vector.tensor_mul(t2[:], dst, stbl[:])
                        for b0 in (0, 64):
                            nc.gpsimd.dma_start(out=t2s[b0:b0 + 32, :],
                                                in_=t2[b0 + 32:b0 + 64, :])
                            nc.gpsimd.dma_start(out=t2s[b0 + 32:b0 + 64, :],
                                                in_=t2[b0:b0 + 32, :])
                        nc.vector.tensor_add(dst, dst, t2s[:])
                        if post is not None:
                            post()
                return run
            return [chunk(0, False), chunk(1, True)]

        def k_fill(pi_):
            kt_ = qk.tile([P, SK], bf16, tag="kt", name="kt", bufs=3)
            kts[pi_] = kt_

            def post():
                nc.sync.dma_start(out=kstg[pi_][:, :], in_=kt_[:, 0:SQ])
                nc.gpsimd.collective_compute(
                    "AllGather", BYP, replica_groups=RG,
                    ins=[kstg[pi_][:, :]], outs=[kgth[pi_][:, :, :]])
            return qk_chunks(wsl[pi_][1], kt_, 0, xs, ck, sk_, xone, post)

        def q_fill(pi_):
            qt_ = qk.tile([P, SQ], bf16, tag="qt", name="qt", bufs=3)
            qts[pi_] = qt_
            return qk_chunks(wsl[pi_][0], qt_, 0, xs, ck, sk_, xone)

        def k_local(pi_):
            # pi0/pi1: project BOTH pair-halves locally in slot order --
            # no collective gates the startup
            kt_ = qk.tile([P, SK], bf16, tag="kt", name="kt", bufs=3)
            qk_proj(wsl[pi_][1], kt_[:, 0:SQ], xa, cka, ska, xaone)
            qk_proj(wsl[pi_][1], kt_[:, SQ:SK], xb, ckb, skb, xbone)
            kts[pi_] = kt_

        def q_stage(pi_):
            qt_ = qk.tile([P, SQ], bf16, tag="qt", name="qt", bufs=3)
            qk_proj(wsl[pi_][0], qt_[:], xs, ck, sk_, xone)
            qts[pi_] = qt_

        def load_ktile(pi_):
            for s_ in range(2):
                nc.sync.dma_start(out=kts[pi_][:, s_ * SQ:(s_ + 1) * SQ],
                                  in_=kgth[pi_][s_, :, :])

        # V for ALL heads and BOTH pair-halves lives in one big SBUF
        # tile, computed locally from xa/xb in global slot order (so it
        # agrees with both the local pi0/pi1 K tiles and the gathered-K
        # slot order). No V collectives exist at all.
        vbig = sb.tile([P, 2, NT // 2, H, HD + 1], bf16, tag="vbig",
                       name="vbig")
        nc.vector.memset(vbig[:, :, :, :, HD:HD + 1], 1.0)

        def v_block(s_, tp_, h0, nh):
            """V proj for heads [h0, h0+nh) x token chunks (2tp_, 2tp_+1)
            of pair-half s_: one u-slot, two nh*64-wide accum groups."""
            xv, xo = (xa, xaone) if s_ == 0 else (xb, xbone)
            hw = nh * HD
            ups = u_slot("vps")
            for ti in range(2):
                t_ = 2 * tp_ + ti
                tqs = slice(ti * hw, (ti + 1) * hw)
                ohs = slice(h0 * HD, h0 * HD + hw)
                for d_ in range(ND):
                    nc.tensor.matmul(
                        ups[:, tqs], xv[d_][:, t_ * P:(t_ + 1) * P],
                        wv[:, d_, ohs],
                        start=(d_ == 0),
                        stop=(not with_bias and d_ == ND - 1))
                if with_bias:
                    nc.tensor.matmul(
                        ups[:, tqs], xo[:, t_ * P:(t_ + 1) * P],
                        wvb[:, ohs], start=False, stop=True)
            nc.vector.tensor_copy(
                vbig[:, s_, 2 * tp_:2 * tp_ + 2, h0:h0 + nh, 0:HD],
                ups[:, 0:2 * hw].rearrange("p (t h d) -> p t h d",
                                           h=nh, d=HD))

        # ---- normalization stage machinery (off the critical path) ----
        pending = []

        def norm_hops(batch):
            for ent in batch:
                den = npool.tile([1, QF], f32, tag="den", name="den", bufs=4)
                nc.gpsimd.dma_start(out=den[:], in_=ent[3][HD:HD + 1, :])
                ent.append(den)

        def norm_recips(batch):
            for ent in batch:
                rc = npool.tile([1, QF], f32, tag="rc", name="rc", bufs=4)
                nc.vector.reciprocal_approx_fast(rc[:], ent[4][:])
                ent.append(rc)

        def norm_bcasts(batch):
            for ent in batch:
                bc = npool.tile([HD, QF], f32, tag="bc", name="bc", bufs=3)
                nc.gpsimd.partition_broadcast(bc[:], ent[5][:])
                ent.append(bc)

        def norm_muls(batch):
            for h, ppi, qqs, osb, den, rc, bc in batch:
                if h % 2 == 0:
                    nc.vector.tensor_mul(
                        at[ppi][0:64, qqs], osb[0:HD, :], bc[:])
                else:
                    atm = npool.tile([HD, QF], bf16, tag="atm", name="atm", bufs=2)
                    nc.vector.tensor_mul(atm[:], osb[0:HD, :], bc[:])
                    nc.gpsimd.dma_start(out=at[ppi][64:128, qqs], in_=atm[:])

        def flush_norm():
            norm_hops(pending)
            norm_recips(pending)
            norm_bcasts(pending)
            norm_muls(pending)
            pending.clear()

        # ---- prologue: V(heads 0..3), local K0/K1, Q0, Q1 --------------
        # nothing before pi0's attention depends on a collective; the CC
        # engine boots (~25us) + runs the pi>=2 K gathers entirely in
        # the shadow of pi0/pi1's attention. V for heads 4..15 is
        # emitted spread across the first bodies (consumed 2+ pis later).
        for s_ in range(2):
            for tp_ in range(4):
                v_block(s_, tp_, 0, 4)
        k_local(0)
        q_stage(0)
        k_local(1)
        q_stage(1)
        vrest = [(s_, tp_, h0, 4) for h0 in (4, 8, 12)
                 for s_ in range(2) for tp_ in range(4)]

        # ---- fused attention + in-stream projection filler -------------
        def qh_block(pi, qh, hook, filler):
            """Emit scores+exp+attn@V for (pi, qh); `hook` (the previous
            block's deferred finisher) runs after kcp0's scores so its
            exp waits hide under fresh matmuls. One `filler` thunk
            (projection chunk / V block) is popped after each odd kcp so
            the exp stream never drains during projection phases.
            Returns this block's own deferred finisher."""
            qt_, kt_ = qts[pi], kts[pi]
            qs = slice(qh * QF, (qh + 1) * QF)
            ope = pso.tile([HD + 1, QF], f32, tag="o", name="o")
            opo = pso.tile([HD + 1, QF], f32, tag="o", name="o")
            pend_e = []

            def attnv_burst(last):
                for bee, beo, bk in pend_e:
                    for j in range(2):
                        kc = 2 * bk + j
                        js = slice(j * QF, (j + 1) * QF)
                        vse = vbig[:, kc // (NT // 2), kc % (NT // 2),
                                   2 * pi, :]
                        vso = vbig[:, kc // (NT // 2), kc % (NT // 2),
                                   2 * pi + 1, :]
                        nc.tensor.matmul(
                            ope[:], vse, bee[:, js],
                            start=(kc == 0), stop=(last and kc == NT - 1))
                        nc.tensor.matmul(
                            opo[:], vso, beo[:, js],
                            start=(kc == 0), stop=(last and kc == NT - 1))
                pend_e.clear()

            for kcp in range(NT // 2):
                spe = u_slot("spe")
                spo = u_slot("spo")
                # even/odd head score MMs adjacent on disjoint PE row
                # groups (h0/h64) and disjoint PSUM banks
                for j in range(2):
                    ks_ = slice((2 * kcp + j) * P, (2 * kcp + j + 1) * P)
                    nc.tensor.matmul(
                        spe[:, j * QF:(j + 1) * QF],
                        kt_[0:64, ks_], qt_[0:64, qs],
                        start=True, stop=True)
                    nc.tensor.matmul(
                        spo[:, j * QF:(j + 1) * QF],
                        kt_[64:128, ks_], qt_[64:128, qs],
                        start=True, stop=True)
                ee = etp.tile([P, SQ], bf16, tag="e", name="e")
                eo = etp.tile([P, SQ], bf16, tag="e", name="e")
                nc.scalar.activation(ee[:], spe[:], AF.Exp, scale=0.125)
                nc.scalar.activation(eo[:], spo[:], AF.Exp, scale=0.125)
                pend_e.append((ee, eo, kcp))
                if kcp == 0 and hook is not None:
                    hook()
                if kcp >= 2 and kcp % 2 == 0:
                    cur = pend_e.pop()
                    attnv_burst(last=False)
                    pend_e.append(cur)
                if kcp % 2 == 1 and filler:
                    th = filler.pop(0)
                    if th is not None:
                        th()

            def finish():
                attnv_burst(last=True)
                for h, op in ((2 * pi, ope), (2 * pi + 1, opo)):
                    osb = npool.tile([HD + 1, QF], f32, tag="osb",
                                     name="osb", bufs=6)
                    nc.vector.tensor_copy(osb[:], op[:])
                    pending.append([h, pi, qs, osb])
            return finish

        def o_qc(qc):
            def run():
                ups = u_slot("ops")
                for oh in range(2):
                    ohs = slice(oh * QF, (oh + 1) * QF)
                    for f in range(ND):
                        nc.tensor.matmul(
                            ups[:, ohs], at[f][:, qc * P:(qc + 1) * P],
                            wo[:, f, ohs],
                            start=(f == 0), stop=(f == ND - 1))
                for oh in range(2):
                    ohs = slice(oh * QF, (oh + 1) * QF)
                    ob = outp.tile([P, QF], bf16, tag="ob", name="ob",
                                   bufs=2)
                    nc.vector.tensor_copy(ob[:], ups[:, ohs])
                    nc.sync.dma_start(out=out[qc * P:(qc + 1) * P, ohs],
                                      in_=ob[:])
            return run

        hook = None
        for pi in range(ND):
            # next-pi K load first: its gather completed a full pi ago
            if 2 <= pi + 1 < ND:
                load_ktile(pi + 1)
            filler = []
            if pi + 2 < ND:
                wsl[pi + 2] = (
                    load_wslice(wqT[:, (pi + 2) * P:(pi + 3) * P], "wq"),
                    load_wslice(wkT[:, (pi + 2) * P:(pi + 3) * P], "wk"))
                filler += k_fill(pi + 2)
                filler += q_fill(pi + 2)
            # body0 takes ALL heads-4..7 V blocks (4 spill past qh1,
            # still ~40us ahead of pi2's attn@V) so pi2 never waits
            for _ in range(8 if pi == 0 else 4):
                if vrest:
                    s_, tp_, h0, nh = vrest.pop(0)
                    filler.append(lambda a=s_, b=tp_, c=h0, d=nh:
                                  v_block(a, b, c, d))
            fin0 = qh_block(pi, 0, hook, filler)

            def hook1(f=fin0):
                f()
                flush_norm()
            fin1 = qh_block(pi, 1, hook1, filler)
            # qh1's finisher is always deferred into the NEXT block's
            # kcp0 (qh0 of pi+1, or o-proj for pi7) for exp runway
            hook = fin1
            # any filler not absorbed by the qh slots lands here
            for th in filler:
                th()
            filler.clear()
            if pi == 5:
                # wo reuses wv's SBUF (tag wbig, bufs=1); emit here so the
                # sync queue has it resident well before o-proj
                wo = wp.tile([P, ND, D], bf16, tag="wbig", name="wo", bufs=1)
                for d_ in range(ND):
                    nc.sync.dma_start(out=wo[:, d_, :],
                                      in_=woT[d_ * P:(d_ + 1) * P, :])

        # ---- o-projection: pi7-qh1's deferred finisher + final norm
        # flush run behind qc0 so qc4..7 find their norms done
        for qc in range(ND):
            if qc == 1 and hook is not None:
                hook()
                flush_norm()
                hook = None
            o_qc(qc)()
    nc.compile()
    return nc


def _rope_tables(pos):
    """pos [n] -> (cos [128, n] bf16, sign-folded TAN [128, n] bf16).

    The kernel computes the sin term as (q*cos) * tan, so the PSUM
    projection tile has a single DVE reader. min |cos| over the table is
    ~6.8e-6 (no bf16 zeros), so q*cos*tan == q*sin to bf16 accuracy."""
    inv = ROPE_BASE ** (-np.arange(0, HD, 2, dtype=np.float64) / HD)
    fr = np.outer(pos.astype(np.float64), inv)          # [n, 32]
    c, s = np.cos(fr), np.sin(fr)
    cos64 = np.concatenate([c, c], axis=1).T            # [64, n]
    tanA = np.concatenate([s / c, -s / c], axis=1).T    # [64, n]
    return (np.tile(cos64, (2, 1)).astype(BF16),
            np.tile(tanA, (2, 1)).astype(BF16))


def _aug_w(w, b):
    """[D, D] weight + [D] bias -> bf16 [D+1, D] (W.T with bias row)."""
    wa = np.empty((D + 1, D), dtype=np.float32)
    wa[:D] = np.asarray(w, dtype=np.float32).T
    wa[D] = np.asarray(b, dtype=np.float32)
    return np.ascontiguousarray(wa).astype(BF16)


def kernel(hidden_states, position_ids, Wq, bq, Wk, bk, Wv, bv, Wo):
    from concourse import bass_utils

    with_bias = bool(
        np.any(np.asarray(bq)) or np.any(np.asarray(bk)) or np.any(np.asarray(bv)))
    key = ("nc", with_bias)
    if key not in _cache:
        _cache[key] = _build_nc(with_bias)
    nc = _cache[key]

    hs = np.asarray(hidden_states, dtype=np.float32)
    pos = np.asarray(position_ids)
    wq = _aug_w(Wq, bq)
    wk = _aug_w(Wk, bk)
    wv = _aug_w(Wv, bv)
    wo = np.ascontiguousarray(np.asarray(Wo, dtype=np.float32).T).astype(BF16)

    in_maps = []
    for core in range(NCORES):
        b, hf = core // 2, core % 2

        def xt_half(h):
            xh = np.empty((D + 1, SQ), dtype=np.float32)
            xh[:D] = hs[b][h * SQ:(h + 1) * SQ].T
            xh[D] = 1.0
            return xh.astype(BF16)

        ck, sk = _rope_tables(np.asarray(pos[b][hf * SQ:(hf + 1) * SQ]))
        cka, ska = _rope_tables(np.asarray(pos[b][0:SQ]))
        ckb, skb = _rope_tables(np.asarray(pos[b][SQ:2 * SQ]))
        in_maps.append({
            "xT": xt_half(hf), "xaT": xt_half(0), "xbT": xt_half(1),
            "wqT": wq, "wkT": wk, "wvT": wv, "woT": wo,
            "cosk": ck, "sink": sk, "cosa": cka, "sina": ska,
            "cosb": ckb, "sinb": skb,
        })

    res = bass_utils.run_bass_kernel_spmd(
        nc, in_maps, core_ids=list(range(NCORES)), trace=TRACE, **TRACE_KW)
    LAST["exec_time_ns"] = res.exec_time_ns
    LAST["mean_exec_time_ns"] = res.mean_exec_time_ns
    LAST["trace"] = res.instructions_and_trace
    LAST["profile_json"] = res.profile_json

    outp_full = np.empty((B, S, D), dtype=np.float32)
    for core in range(NCORES):
        b, hf = core // 2, core % 2
        outp_full[b, hf * SQ:(hf + 1) * SQ] = np.asarray(
            res.results[core]["out"], dtype=np.float32)
    return outp_full
